# revision 2
# baseline (speedup 1.0000x reference)
"""Trainium2 Bass kernel for nn_DenseGraphConvNodeToEdge — fp8 DoubleRow.

out[b,i,j,o] = y_rows[b,i,o] + base[b,j,o]
  base = y_cols + y_sum + bias;  y_rows = x @ W1.T

v2 strategy vs baseline: the baseline produced every output element on
the PE via K=65 bf16 matmuls (1 col/cycle -> ~219us PE floor, the
wall). Here the big matmuls run in fp8e4 (e4m3) with
MatmulPerfMode.DoubleRow: the PE consumes 2 fp8 columns/cycle
(cycles_per_row=0.5), halving the PE time to ~110us and moving the
wall to the 64 MiB/core output DMA (~187us at ~358 GB/s).

K layout (DoubleRow contracts 2 groups of 64):
  group i=0: rows k=0..63  = x channels (fp8), rhs = W1rep (W1.T tiled
             128x along free: 8192 = 128 j x 64 o)
  group i=1: rows k=0,1,2  = ones; rhs rows = base_hi/mid/lo, the
             fp8 3-way split of base[b,j,o] (flattened per jblk).
             rows 3..63 zero.
fp8 error budget: x/W1 quantization ~0.17 abs worst, 3-way base split
residual <= ~0.02, bf16 output step 0.125 at |out|~64 -> ~5e-3 rel of
max (gate 2e-2).

base[b,j,o] is computed exactly as the baseline: K=65 bf16 matmul
(x row + ones) x (W0.T ; s2row) where s2row = x.sum(1) @ W2.T + bias
in exact f32. Split chain per jblk: hi=fp8(ps) [ACT], r2=ps-hi f32
[DVE], mid=fp8(r2) [Pool], lo=fp8(r2-mid) [Pool]; 3 flatten DMAs
(gpsimd SWDGE) rewrite rhs rows 0..2 of the i=1 half.

PSUM->SBUF bf16 copies split ACT:DVE = 5:4 (153.6 vs 123 G elem/s).
Output DMAs alternate sync/scalar HWDGE rings; last jblks drain as
per-group DMAs split across both rings.
"""

import numpy as np

B, N, C = 4, 1024, 64
N_CORES = 8
R = N // N_CORES  # 128 rows per core

_CACHE = {}


def _build():
    import concourse.tile as tile
    from concourse import bacc, mybir

    f32 = mybir.dt.float32
    bf16 = mybir.dt.bfloat16
    fp8 = mybir.dt.float8e4
    DR = mybir.MatmulPerfMode.DoubleRow

    nc = bacc.Bacc("TRN2", target_bir_lowering=False, debug=False,
                   num_devices=N_CORES)

    xt1b = nc.dram_tensor("xt1b", [C + 1, B * N], bf16, kind="ExternalInput").ap()
    xq = nc.dram_tensor("xq", [C, 2 * B * R], fp8, kind="ExternalInput").ap()
    w1q = nc.dram_tensor("w1q", [C, 2 * 8192], fp8, kind="ExternalInput").ap()
    w0tb = nc.dram_tensor("w0tb", [C, C], bf16, kind="ExternalInput").ap()
    w2t = nc.dram_tensor("w2t", [C, C], f32, kind="ExternalInput").ap()
    bias_row = nc.dram_tensor("bias_row", [1, C], f32, kind="ExternalInput").ap()
    out_s = nc.dram_tensor("out_s", [B, R, N, C], bf16, kind="ExternalOutput").ap()

    with tile.TileContext(nc) as tc:
        with (
            tc.tile_pool(name="const", bufs=1) as const_pool,
            tc.tile_pool(name="rhs", bufs=1) as rhs_pool,
            tc.tile_pool(name="base", bufs=8) as base_pool,
            tc.tile_pool(name="stage", bufs=3) as stage_pool,
            tc.tile_pool(name="psm", bufs=3, space="PSUM") as psum_main,
            tc.tile_pool(name="pss", bufs=2, space="PSUM") as psum_small,
        ):
            # ---- persistent SBUF state ----
            xt1_bf = const_pool.tile([C + 1, B * N], bf16, tag="xt1b")
            lhsT_sb = const_pool.tile([C, 2 * B * R], fp8, tag="lhsT")
            rhs_base = const_pool.tile([C + 1, C], bf16, tag="rhsb")
            w2t_sb = const_pool.tile([C, C], f32, tag="w2t")
            bias_sb = const_pool.tile([1, C], f32, tag="bias")
            xsum_sb = const_pool.tile([C, 1], f32, tag="xsum")
            rhs_bufs = [rhs_pool.tile([C, 2 * 8192], fp8, tag=f"rhs{k}",
                                      name=f"rhs{k}")
                        for k in range(3)]

            # ---- input DMAs, spread across the three DGE rings ----
            nc.sync.dma_start(xt1_bf[:, 0:N], xt1b[:, 0:N])
            nc.sync.dma_start(xt1_bf[:, N:B * N], xt1b[:, N:B * N])
            nc.scalar.dma_start(rhs_base[0:C, :], w0tb[:, :])
            nc.scalar.dma_start(rhs_bufs[0][:, :], w1q[:, :])
            nc.scalar.dma_start(rhs_bufs[1][:, :], w1q[:, :])
            nc.scalar.dma_start(rhs_bufs[2][:, :], w1q[:, :])
            nc.gpsimd.dma_start(w2t_sb[:], w2t[:, :])
            nc.gpsimd.dma_start(bias_sb[:], bias_row[:, :])
            nc.gpsimd.dma_start(lhsT_sb[:], xq[:, :])

            # [C, 2, B, R] view of the DoubleRow stationary tensor
            lhsT_4d = lhsT_sb[:, :].rearrange("p (i b m) -> p i b m", i=2, b=B)

            copy_idx = 0  # 5:4 ACT:DVE split for PSUM->SBUF copies
            for b in range(B):
                # xsum[c] = sum_j x[b,j,c] (bf16 in, f32 accumulate)
                nc.vector.reduce_sum(
                    xsum_sb[:], xt1_bf[0:C, b * N:(b + 1) * N],
                    axis=mybir.AxisListType.X)
                # s2_row[o] = sum_c xsum[c] * W2[o,c] + bias[o] (exact fp32)
                ps_s2 = psum_small.tile([1, C], f32, tag="pss")
                nc.tensor.matmul(ps_s2[:], xsum_sb[:], w2t_sb[:],
                                 start=True, stop=True)
                nc.vector.tensor_add(rhs_base[C:C + 1, :], ps_s2[:], bias_sb[:])

                # precompute all 8 (hi, mid, lo) base splits for this b
                base_tiles = []
                for jblk in range(8):
                    # base tile [128 j, 64 o] (bf16 GEMM, f32 accumulate)
                    ps_b = psum_small.tile([128, C], f32, tag="pss")
                    nc.tensor.matmul(
                        ps_b[:],
                        xt1_bf[:, b * N + jblk * 128: b * N + (jblk + 1) * 128],
                        rhs_base[:],
                        start=True, stop=True)
                    hi = base_pool.tile([128, C], fp8, tag="bhi",
                                        name=f"hi_{b}_{jblk}")
                    r2 = base_pool.tile([128, C], f32, tag="br2",
                                        name=f"r2_{b}_{jblk}")
                    mid = base_pool.tile([128, C], fp8, tag="bmid",
                                         name=f"mid_{b}_{jblk}")
                    lo = base_pool.tile([128, C], fp8, tag="blo",
                                        name=f"lo_{b}_{jblk}")
                    nc.scalar.copy(hi[:], ps_b[:])
                    nc.vector.tensor_sub(r2[:], ps_b[:], hi[:])
                    nc.gpsimd.tensor_copy(mid[:], r2[:])
                    nc.gpsimd.tensor_sub(lo[:], r2[:], mid[:])
                    base_tiles.append((hi, mid, lo))

                lhsT = lhsT_4d[:, :, b, :]
                for jblk in range(8):
                    rhs = rhs_bufs[(b * 8 + jblk) % 3]
                    hi, mid, lo = base_tiles[jblk]
                    # flatten [128 j, 64 o] -> rows 0..2 of the i=1 half
                    if b == 0 and jblk < 2:
                        # startup-latency-critical: split each flatten into
                        # two concurrent 64-packet DMAs on separate rings
                        for row, src in ((0, hi), (1, mid), (2, lo)):
                            dst = rhs[row:row + 1, 8192:2 * 8192]
                            nc.gpsimd.dma_start(
                                dst[:, 0:4096].rearrange(
                                    "a (p o) -> a p o", p=64),
                                src[0:64, :])
                            nc.sync.dma_start(
                                dst[:, 4096:8192].rearrange(
                                    "a (p o) -> a p o", p=64),
                                src[64:128, :])
                    else:
                        for row, src in ((0, hi), (1, mid), (2, lo)):
                            nc.gpsimd.dma_start(
                                rhs[row:row + 1, 8192:2 * 8192].rearrange(
                                    "a (p o) -> a p o", p=128),
                                src[:])
                    rhs3 = rhs[:, :].rearrange("p (i n) -> p i n", i=2)
                    # main GEMMs: 16 x [64,2,512] fp8 DoubleRow matmuls
                    last = (b == B - 1 and jblk >= 6)
                    stage_t = stage_pool.tile([128, 8192], bf16, tag="stage")
                    j0 = jblk * 128
                    for g in range(8):  # psum groups of [128, 1024]
                        ps_m = psum_main.tile([128, 1024], f32, tag="psm")
                        for h in range(2):
                            c0 = g * 1024 + h * 512
                            nc.tensor.matmul(
                                ps_m[:, h * 512:(h + 1) * 512],
                                lhsT,
                                rhs3[:, :, c0:c0 + 512],
                                start=True, stop=True, perf_mode=DR)
                        dst = stage_t[:, g * 1024:(g + 1) * 1024]
                        if copy_idx % 9 in (0, 2, 4, 6, 8):
                            nc.scalar.copy(dst, ps_m[:])
                        else:
                            nc.vector.tensor_copy(dst, ps_m[:])
                        copy_idx += 1
                        if last:
                            # drain the final jblks as per-group DMAs split
                            # by partition halves across both rings
                            js = slice(j0 + g * 16, j0 + (g + 1) * 16)
                            nc.sync.dma_start(
                                out_s[b, 0:64, js, :], dst[0:64, :])
                            nc.scalar.dma_start(
                                out_s[b, 64:128, js, :], dst[64:128, :])
                    if not last:
                        dma_eng = nc.sync if (b * 8 + jblk) % 2 == 0 else nc.scalar
                        dma_eng.dma_start(out_s[b, :, j0:j0 + 128, :], stage_t[:])

    nc.compile()
    return nc


def _get_nc():
    if "nc" not in _CACHE:
        _CACHE["nc"] = _build()
    return _CACHE["nc"]


def _pack_inputs(x, W0, W1, W2, bias):
    import ml_dtypes

    bf = ml_dtypes.bfloat16
    f8 = ml_dtypes.float8_e4m3
    x = np.ascontiguousarray(np.asarray(x, dtype=np.float32))
    W0 = np.asarray(W0, dtype=np.float32)
    W1 = np.asarray(W1, dtype=np.float32)
    W2 = np.asarray(W2, dtype=np.float32)
    bias = np.asarray(bias, dtype=np.float32)

    ones_n = np.ones((B, 1, N), dtype=np.float32)
    xt1b = np.ascontiguousarray(np.concatenate(
        [x.transpose(0, 2, 1), ones_n], axis=1).transpose(1, 0, 2)
        .reshape(C + 1, B * N).astype(bf))

    w1q = np.zeros((C, 2, 8192), dtype=f8)
    w1q[:, 0, :] = np.tile(W1.T.astype(f8), (1, 128))
    w1q = np.ascontiguousarray(w1q.reshape(C, 2 * 8192))

    w0tb = np.ascontiguousarray(W0.T.astype(bf))
    w2t = np.ascontiguousarray(W2.T)
    bias_row = np.ascontiguousarray(bias.T)

    in_maps = []
    for c in range(N_CORES):
        xr = x[:, c * R:(c + 1) * R, :]  # [B, R, C]
        xq = np.zeros((C, 2, B, R), dtype=f8)
        xq[:, 0] = xr.transpose(2, 0, 1).astype(f8)
        xq[0, 1] = 1.0
        xq[1, 1] = 1.0
        xq[2, 1] = 1.0
        xq = np.ascontiguousarray(xq.reshape(C, 2 * B * R))
        in_maps.append({
            "xt1b": xt1b, "xq": xq, "w1q": w1q,
            "w0tb": w0tb, "w2t": w2t, "bias_row": bias_row,
        })
    return in_maps


def kernel(x, adj, W0, W1, W2, bias):
    from concourse.bass_utils import run_bass_kernel_spmd

    nc = _get_nc()
    in_maps = _pack_inputs(x, W0, W1, W2, bias)

    global _last_in_maps
    _last_in_maps = in_maps
    res = run_bass_kernel_spmd(nc, in_maps, list(range(N_CORES)))

    out = np.empty((B, N, N, C), dtype=np.float32)
    for c in range(N_CORES):
        out[:, c * R:(c + 1) * R] = np.asarray(
            res.results[c]["out_s"]).astype(np.float32)
    return out


# revision 3
# speedup vs baseline: 1.2740x; 1.2740x over previous
"""Trainium2 Bass kernel for nn_DenseGraphConvNodeToEdge — v3.

out[b,i,j,o] = y_rows[b,i,o] + base[b,j,o]
  base = y_cols + y_sum + bias;  y_rows = x @ W1.T

The PE (this LNC config) streams 512-col bf16 matmuls at a fixed
427ns (1.2 G cols/s = 153.6 G elem/s) regardless of dtype tricks
(fp8 DoubleRow measured: same column rate), so producing all 33.5M
elems/core on the PE floors at ~219us while the 64 MiB output DMA
floors at ~190us. v3 therefore offloads OFF_JBLKS of the 32 j-blocks
per core to a second producer pipeline:

  PE jblk  : K=65 bf16 matmul (x rows + ones) x (W1rep ; base row),
             PSUM -> SBUF bf16 copies on ACT:DVE = 3:2.
  OFF jblk : flatten base -> [1,8192] row (gpsimd SWDGE), gpsimd
             partition_broadcast -> [128,8192] (12us, Pool otherwise
             idle), DVE tensor_add with y_rows free-dim stride-0
             broadcast ([128,64] read as [128,128j,64o]) at 2x mode
             (~4.4us) straight into the staging tile.

Other changes vs the 266us baseline (all from trace evidence):
  * bias folded into w2tb row 64 (the 256B bias DMA's 16B packets
    completed at t=18us and gated the whole first-flatten chain).
  * rhs_base double-buffered over b and prep(b+1) emitted at jblk 6
    of b, so the s2/y_rows/base-tile chain of the next batch overlaps
    the current batch's main matmuls.
  * startup DMAs ordered by need; first two flattens split across
    gpsimd+sync rings (packet-latency-bound).

Output staged bf16 (rel ~2^-9; gate is 2e-2) -> 64 MiB/core writes.
"""

import numpy as np

B, N, C = 4, 1024, 64
N_CORES = 8
R = N // N_CORES  # 128 rows per core

# global jblk ids (b*8+jblk) produced by the broadcast+DVE pipeline.
# Keep out of {28..31} (per-group tail drain) and spread across b.
OFF_JBLKS = frozenset({4, 9, 14, 19, 22, 25, 27})

_CACHE = {}


def _build():
    import concourse.tile as tile
    from concourse import bacc, mybir

    f32 = mybir.dt.float32
    bf16 = mybir.dt.bfloat16

    nc = bacc.Bacc("TRN2", target_bir_lowering=False, debug=False,
                   num_devices=N_CORES)

    xt1b = nc.dram_tensor("xt1b", [C + 1, B * N], bf16, kind="ExternalInput").ap()
    xrt1b = nc.dram_tensor("xrt1b", [C + 1, B * R], bf16, kind="ExternalInput").ap()
    w1w = nc.dram_tensor("w1w", [C, 8192], bf16, kind="ExternalInput").ap()
    w0tb = nc.dram_tensor("w0tb", [C, C], bf16, kind="ExternalInput").ap()
    w1tb = nc.dram_tensor("w1tb", [C, C], bf16, kind="ExternalInput").ap()
    w2tb = nc.dram_tensor("w2tb", [C + 1, C], f32, kind="ExternalInput").ap()
    out_s = nc.dram_tensor("out_s", [B, R, N, C], bf16, kind="ExternalOutput").ap()

    with tile.TileContext(nc) as tc:
        with (
            tc.tile_pool(name="const", bufs=1) as const_pool,
            tc.tile_pool(name="rhs", bufs=1) as rhs_pool,
            tc.tile_pool(name="base", bufs=16) as base_pool,
            tc.tile_pool(name="yr", bufs=2) as yr_pool,
            tc.tile_pool(name="row", bufs=2) as row_pool,
            tc.tile_pool(name="bc", bufs=2) as bc_pool,
            tc.tile_pool(name="stage", bufs=3) as stage_pool,
            tc.tile_pool(name="psm", bufs=3, space="PSUM") as psum_main,
            tc.tile_pool(name="pss", bufs=2, space="PSUM") as psum_small,
        ):
            # ---- persistent SBUF state ----
            xt1_bf = const_pool.tile([C + 1, B * N], bf16, tag="xt1b")
            lhsT_sb = const_pool.tile([C + 1, B * R], bf16, tag="lhsT")
            rhs_base = [const_pool.tile([C + 1, C], bf16, tag=f"rhsb{k}",
                                        name=f"rhsb{k}")
                        for k in range(2)]
            w1t_sb = const_pool.tile([C, C], bf16, tag="w1t")
            w2t_sb = const_pool.tile([C + 1, C], f32, tag="w2t")
            xsum_sb = const_pool.tile([C + 1, 1], f32, tag="xsum")
            rhs_bufs = [rhs_pool.tile([C + 1, 8192], bf16, tag=f"rhs{k}",
                                      name=f"rhs{k}")
                        for k in range(3)]

            # xsum row 64 = 1.0 so the s2 matmul picks up the bias row of
            # w2tb; DVE is idle at startup
            nc.vector.memset(xsum_sb[C:C + 1, :], 1.0)

            # ---- input DMAs, ordered by first use on each ring ----
            nc.sync.dma_start(xt1_bf[:, 0:N], xt1b[:, 0:N])
            nc.sync.dma_start(xt1_bf[:, N:B * N], xt1b[:, N:B * N])
            nc.scalar.dma_start(rhs_base[0][0:C, :], w0tb[:, :])
            nc.scalar.dma_start(rhs_bufs[0][:C, :], w1w[:, :])
            nc.scalar.dma_start(rhs_bufs[1][:C, :], w1w[:, :])
            nc.scalar.dma_start(rhs_bufs[2][:C, :], w1w[:, :])
            nc.scalar.dma_start(rhs_base[1][0:C, :], w0tb[:, :])
            nc.gpsimd.dma_start(w2t_sb[:], w2tb[:, :])
            nc.gpsimd.dma_start(lhsT_sb[:], xrt1b[:, :])
            nc.gpsimd.dma_start(w1t_sb[:], w1tb[:, :])

            base_tiles = {}
            yrows = {}

            def emit_prep(b):
                rb = rhs_base[b % 2]
                # xsum[c] = sum_j x[b,j,c] (bf16 in, f32 accumulate)
                nc.vector.reduce_sum(
                    xsum_sb[0:C, :], xt1_bf[0:C, b * N:(b + 1) * N],
                    axis=mybir.AxisListType.X)
                # s2_row[o] = xsum @ W2.T + bias (exact fp32, bias in row 64)
                ps_s2 = psum_small.tile([1, C], f32, tag="pss")
                nc.tensor.matmul(ps_s2[:], xsum_sb[:], w2t_sb[:],
                                 start=True, stop=True)
                nc.vector.tensor_copy(rb[C:C + 1, :], ps_s2[:])
                # y_rows[b] = x_r @ W1.T  [128 i, 64 o] bf16
                ps_y = psum_small.tile([128, C], f32, tag="pss")
                nc.tensor.matmul(
                    ps_y[:], lhsT_sb[0:C, b * R:(b + 1) * R], w1t_sb[:],
                    start=True, stop=True)
                yt = yr_pool.tile([128, C], bf16, tag="yr", name=f"yr_{b}")
                nc.scalar.copy(yt[:], ps_y[:])
                yrows[b] = yt
                for jblk in range(8):
                    ps_b = psum_small.tile([128, C], f32, tag="pss")
                    nc.tensor.matmul(
                        ps_b[:],
                        xt1_bf[:, b * N + jblk * 128: b * N + (jblk + 1) * 128],
                        rb[:],
                        start=True, stop=True)
                    bt = base_pool.tile([128, C], bf16, tag="base",
                                        name=f"base_{b}_{jblk}")
                    if jblk % 2 == 0:
                        nc.vector.tensor_copy(bt[:], ps_b[:])
                    else:
                        nc.scalar.copy(bt[:], ps_b[:])
                    base_tiles[(b, jblk)] = bt

            copy_idx = 0   # 3:2 ACT:DVE over PSUM->SBUF copies
            rhs_idx = 0    # rhs buffer rotation over PE-jblks only
            for b in range(B):
                if b == 0:
                    emit_prep(0)
                lhsT = lhsT_sb[:, b * R:(b + 1) * R]
                for jblk in range(8):
                    if jblk == 6 and b + 1 < B:
                        emit_prep(b + 1)
                    g = b * 8 + jblk
                    j0 = jblk * 128
                    bt = base_tiles.pop((b, jblk))
                    last = g >= 30

                    if g in OFF_JBLKS:
                        # broadcast+DVE producer: flatten -> row [1,8192],
                        # Pool partition_broadcast, DVE stride-0 add
                        row_t = row_pool.tile([1, 8192], bf16, tag="row")
                        nc.gpsimd.dma_start(
                            row_t[:, :].rearrange("a (p o) -> a p o", p=128),
                            bt[:])
                        bc_t = bc_pool.tile([128, 8192], bf16, tag="bc")
                        nc.gpsimd.partition_broadcast(bc_t[:, :], row_t[:, :])
                        stage_t = stage_pool.tile([128, 8192], bf16,
                                                  tag="stage")
                        y_b = yrows[b][:, :].unsqueeze(1).broadcast_to(
                            (128, 128, C))
                        nc.vector.tensor_add(
                            stage_t[:, :].rearrange("p (j o) -> p j o", j=128),
                            bc_t[:, :].rearrange("p (j o) -> p j o", j=128),
                            y_b)
                        dma_eng = nc.sync if g % 2 == 0 else nc.scalar
                        dma_eng.dma_start(out_s[b, :, j0:j0 + 128, :],
                                          stage_t[:])
                        continue

                    rhs = rhs_bufs[rhs_idx % 3]
                    rhs_idx += 1
                    row64 = rhs[C:C + 1, :]
                    if g < 2:
                        # startup-latency-critical: halve the flatten into
                        # two concurrent 64-packet DMAs
                        nc.gpsimd.dma_start(
                            row64[:, 0:4096].rearrange("a (p o) -> a p o",
                                                       p=64),
                            bt[0:64, :])
                        nc.sync.dma_start(
                            row64[:, 4096:8192].rearrange("a (p o) -> a p o",
                                                          p=64),
                            bt[64:128, :])
                    else:
                        nc.gpsimd.dma_start(
                            row64[:, :].rearrange("a (p o) -> a p o", p=128),
                            bt[:])
                    stage_t = stage_pool.tile([128, 8192], bf16, tag="stage")
                    for grp in range(8):  # psum groups of [128, 1024]
                        ps_m = psum_main.tile([128, 1024], f32, tag="psm")
                        for h in range(2):
                            nc.tensor.matmul(
                                ps_m[:, h * 512:(h + 1) * 512],
                                lhsT,
                                rhs[:, grp * 1024 + h * 512:
                                    grp * 1024 + (h + 1) * 512],
                                start=True, stop=True)
                        dst = stage_t[:, grp * 1024:(grp + 1) * 1024]
                        if copy_idx % 5 in (0, 2, 4):
                            nc.scalar.copy(dst, ps_m[:])
                        else:
                            nc.vector.tensor_copy(dst, ps_m[:])
                        copy_idx += 1
                        if last:
                            # drain the final jblks as per-group DMAs split
                            # by partition halves across both rings
                            js = slice(j0 + grp * 16, j0 + (grp + 1) * 16)
                            nc.sync.dma_start(
                                out_s[b, 0:64, js, :], dst[0:64, :])
                            nc.scalar.dma_start(
                                out_s[b, 64:128, js, :], dst[64:128, :])
                    if not last:
                        dma_eng = nc.sync if g % 2 == 0 else nc.scalar
                        dma_eng.dma_start(out_s[b, :, j0:j0 + 128, :],
                                          stage_t[:])

    nc.compile()
    return nc


def _get_nc():
    if "nc" not in _CACHE:
        _CACHE["nc"] = _build()
    return _CACHE["nc"]


def _pack_inputs(x, W0, W1, W2, bias):
    import ml_dtypes

    bf = ml_dtypes.bfloat16
    x = np.ascontiguousarray(np.asarray(x, dtype=np.float32))
    W0 = np.asarray(W0, dtype=np.float32)
    W1 = np.asarray(W1, dtype=np.float32)
    W2 = np.asarray(W2, dtype=np.float32)
    bias = np.asarray(bias, dtype=np.float32)

    ones_n = np.ones((B, 1, N), dtype=np.float32)
    xt1b = np.ascontiguousarray(np.concatenate(
        [x.transpose(0, 2, 1), ones_n], axis=1).transpose(1, 0, 2)
        .reshape(C + 1, B * N).astype(bf))
    w1w = np.ascontiguousarray(np.tile(W1.T.astype(bf), (1, 128)))
    w0tb = np.ascontiguousarray(W0.T.astype(bf))
    w1tb = np.ascontiguousarray(W1.T.astype(bf))
    w2tb = np.ascontiguousarray(
        np.concatenate([W2.T, bias.T], axis=0))  # [65, 64] f32

    in_maps = []
    ones_r = np.ones((B, 1, R), dtype=np.float32)
    for c in range(N_CORES):
        xr = x[:, c * R:(c + 1) * R, :]
        xrt1b = np.ascontiguousarray(np.concatenate(
            [xr.transpose(0, 2, 1), ones_r], axis=1).transpose(1, 0, 2)
            .reshape(C + 1, B * R).astype(bf))
        in_maps.append({
            "xt1b": xt1b, "xrt1b": xrt1b, "w1w": w1w,
            "w0tb": w0tb, "w1tb": w1tb, "w2tb": w2tb,
        })
    return in_maps


def kernel(x, adj, W0, W1, W2, bias):
    from concourse.bass_utils import run_bass_kernel_spmd

    nc = _get_nc()
    in_maps = _pack_inputs(x, W0, W1, W2, bias)

    global _last_in_maps
    _last_in_maps = in_maps
    res = run_bass_kernel_spmd(nc, in_maps, list(range(N_CORES)))

    out = np.empty((B, N, N, C), dtype=np.float32)
    for c in range(N_CORES):
        out[:, c * R:(c + 1) * R] = np.asarray(
            res.results[c]["out_s"]).astype(np.float32)
    return out
